# revision 21
# baseline (speedup 1.0000x reference)
"""Trainium2 Bass kernel for BarlowTwinsLoss (nn_BarlowTwinsLoss_11038065951192).

Full inputs: e_q, tau [16384, 2048] f32. Output: scalar f32 loss.

Strategy (data-parallel over the batch axis, 8 NeuronCores, NO collective):
  - each core holds a [2048, 2048] row-shard of e_q and tau
  - one pass over the shard computes 5 per-feature partial sums in PSUM via
    ones-vector matmuls: S1e=sum(e), S1t=sum(t), S2e=sum(e^2), S2t=sum(t^2),
    Set=sum(e*t) (stat s lands at PSUM partition 32*(s//2), col half s%2)
  - each core DMAs its raw [5, 2048] partial sums to the output; the host
    sums the 8 cores' partials in float64 and runs the tiny mean/var/corr
    epilogue (the gather/unshard step)

Rationale: the predecessor used an on-device ReduceScatter of the 5x2048
stats; an 8-rank ring RS pays ~10us/step of ncfw control-plane latency x 7
steps (~75us) for a 40KB message -- more than half the kernel's runtime.
Host-side reduction of 8x40KB removes it entirely. The per-core pipeline is
DMA-bound (32MB of f32 loads per core); DVE/ACT/PE all fit under the DMA
shadow. A/Bs that measured WORSE and were reverted: 2MB two-row-tile loads
(+10us), t-loads on the scalar HWDGE ring (+15us), round-robin matmul
col-group interleave (no change).

Hardware pitfalls baked into this design (probed on silicon):
  - DVE tensor_tensor with f32 inputs and bf16 output produces garbage ->
    multiply the bf16 copies instead
  - InstTensorTensorReduce crashes the exec unit -> tensor_mul + matmul
  - ACT reading bf16 input crashes the exec unit -> ACT squares read f32
  - DMA cannot read PSUM -> stage through SBUF with DVE/ACT copies
  - a DMA whose SBUF-side AP merges the partition dim into the free axis
    compiles in bass but fails neuronxcc NEFF codegen -> plain row DMAs

The module is self-contained: it builds + compiles the Bass graph on first
call and caches the jitted PJRT executable for repeat calls.
"""

import numpy as np

N_FULL = 16384
D = 2048
N_CORES = 8
N_SHARD = N_FULL // N_CORES  # 2048 rows per core
P = 128
N_TILES = N_SHARD // P  # 16
CHUNK = 512
N_CHUNKS = D // CHUNK  # 4
NSTATS = 5  # S1e, S1t, S2e, S2t, Set
EPS = 1e-9

_CACHE = {}


def _build_nc(repeat=1, loop=None, mm_order="grouped", tpb=1, io_bufs=3,
              dual_ring=False, body_mode="full", split_last=False):
    import contextlib

    import concourse.bacc as bacc
    import concourse.tile as tile
    from concourse import mybir

    f32 = mybir.dt.float32
    bf16 = mybir.dt.bfloat16
    Act = mybir.ActivationFunctionType

    nc = bacc.Bacc(
        "TRN2",
        target_bir_lowering=False,
        debug=False,
        enable_asserts=False,
        num_devices=1,
    )
    eq_d = nc.dram_tensor("e_q", [N_SHARD, D], f32, kind="ExternalInput")
    ta_d = nc.dram_tensor("tau", [N_SHARD, D], f32, kind="ExternalInput")
    out_d = nc.dram_tensor("out", [NSTATS, D], f32, kind="ExternalOutput")

    with tile.TileContext(nc) as tc:
        with (
            tc.tile_pool(name="io", bufs=io_bufs) as io,
            tc.tile_pool(name="bfp", bufs=2) as bfp,
            tc.tile_pool(name="misc", bufs=1) as misc,
            tc.tile_pool(name="ep", bufs=1) as ep,
            tc.tile_pool(name="psp", bufs=1, space="PSUM") as psp,
        ):
            ones_bf = misc.tile([P, 1], bf16)
            nc.gpsimd.memset(ones_bf[:], 1.0)
            zero_b = misc.tile([P, 1], f32)
            nc.gpsimd.memset(zero_b[:], 0.0)

            pre_e = pre_t = None
            if body_mode == "compute_only":
                # timing probe: compute reads fixed pre-initialized tiles
                pre_e = misc.tile([P, tpb * D], f32, tag="pre_e")
                pre_t = misc.tile([P, tpb * D], f32, tag="pre_t")
                nc.gpsimd.memset(pre_e[:], 0.5)
                nc.gpsimd.memset(pre_t[:], 0.25)

            # stats accumulate in PSUM rows {0,32,64}; untouched partitions
            # are never read (the out-DMAs below address only those 3 rows)
            psum_stats = psp.tile([65, 2 * N_CHUNKS * CHUNK], f32, tag="stats")

            for _rep in range(repeat):
                loop_cm = (
                    tc.For_i(
                        0,
                        loop,
                        1,
                        hint_engines=(
                            mybir.EngineType.PE,
                            mybir.EngineType.DVE,
                            mybir.EngineType.Activation,
                            mybir.EngineType.SP,
                        ),
                    )
                    if loop is not None
                    else contextlib.nullcontext()
                )
                with contextlib.ExitStack() as _stack:
                    _stack.enter_context(loop_cm)

                    W = tpb * D
                    n_it = N_TILES // tpb
                    n_main = n_it - 1 if split_last else n_it
                    for i in range(n_main):
                        if body_mode == "compute_only":
                            e_t, t_t = pre_e, pre_t
                        else:
                            e_t = io.tile([P, W], f32, tag="e")
                            t_t = io.tile([P, W], f32, tag="t")
                        t_dma = nc.scalar if dual_ring else nc.sync
                        if body_mode != "compute_only":
                            if tpb == 1:
                                nc.sync.dma_start(
                                    e_t[:], eq_d[i * P : (i + 1) * P, :]
                                )
                                t_dma.dma_start(
                                    t_t[:], ta_d[i * P : (i + 1) * P, :]
                                )
                            else:
                                rows = slice(i * P * tpb, (i + 1) * P * tpb)
                                nc.sync.dma_start(
                                    e_t[:].rearrange("p (a d) -> p a d", a=tpb),
                                    eq_d[rows, :].rearrange(
                                        "(a p) d -> p a d", p=P
                                    ),
                                )
                                t_dma.dma_start(
                                    t_t[:].rearrange("p (a d) -> p a d", a=tpb),
                                    ta_d[rows, :].rearrange(
                                        "(a p) d -> p a d", p=P
                                    ),
                                )
                        if body_mode == "dma_only":
                            continue

                        e_bf = bfp.tile([P, W], bf16, tag="e_bf")
                        t_bf = bfp.tile([P, W], bf16, tag="t_bf")
                        e2_bf = bfp.tile([P, W], bf16, tag="e2_bf")
                        t2_bf = bfp.tile([P, W], bf16, tag="t2_bf")
                        et_bf = bfp.tile([P, W], bf16, tag="et_bf")

                        nc.vector.tensor_copy(e_bf[:], e_t[:])
                        nc.vector.tensor_copy(t_bf[:], t_t[:])
                        nc.scalar.activation(
                            e2_bf[:], e_t[:], Act.Square, bias=zero_b[:]
                        )
                        nc.scalar.activation(
                            t2_bf[:], t_t[:], Act.Square, bias=zero_b[:]
                        )
                        nc.vector.tensor_mul(et_bf[:], e_bf[:], t_bf[:])

                        srcs = (e_bf, t_bf, e2_bf, t2_bf, et_bf)
                        if mm_order == "grouped":
                            order = [(s, a, c) for s in range(NSTATS)
                                     for a in range(tpb)
                                     for c in range(N_CHUNKS)]
                        else:  # "rr": round-robin PSUM col groups so adjacent
                            # matmuls hit distinct array col-strips (concurrent)
                            order = [(s, a, c) for a in range(tpb)
                                     for c in range(N_CHUNKS)
                                     for s in (0, 2, 4, 1, 3)]
                        for s, a, c in order:
                            g, sl = divmod(s, 2)
                            col = (sl * N_CHUNKS + c) * CHUNK
                            nc.tensor.matmul(
                                psum_stats[
                                    32 * g : 32 * g + 1, col : col + CHUNK
                                ],
                                ones_bf[:, 0:1],
                                srcs[s][
                                    :, a * D + c * CHUNK : a * D + (c + 1) * CHUNK
                                ],
                                start=(i == 0 and a == 0),
                                stop=(i == n_it - 1 and a == tpb - 1),
                            )

                    if split_last:
                        # process the last row-tile in feature chunks so the
                        # post-last-load tail chain covers 512 cols, not 2048
                        assert tpb == 1 and body_mode == "full"
                        rows = slice((N_TILES - 1) * P, N_TILES * P)
                        for c in range(N_CHUNKS):
                            colr = slice(c * CHUNK, (c + 1) * CHUNK)
                            e_c = io.tile([P, CHUNK], f32, tag="e_sc")
                            t_c = io.tile([P, CHUNK], f32, tag="t_sc")
                            nc.sync.dma_start(e_c[:], eq_d[rows, colr])
                            nc.sync.dma_start(t_c[:], ta_d[rows, colr])
                            eb = bfp.tile([P, CHUNK], bf16, tag="eb_sc")
                            tb = bfp.tile([P, CHUNK], bf16, tag="tb_sc")
                            e2b = bfp.tile([P, CHUNK], bf16, tag="e2_sc")
                            t2b = bfp.tile([P, CHUNK], bf16, tag="t2_sc")
                            etb = bfp.tile([P, CHUNK], bf16, tag="et_sc")
                            nc.vector.tensor_copy(eb[:], e_c[:])
                            nc.vector.tensor_copy(tb[:], t_c[:])
                            nc.scalar.activation(
                                e2b[:], e_c[:], Act.Square, bias=zero_b[:]
                            )
                            nc.scalar.activation(
                                t2b[:], t_c[:], Act.Square, bias=zero_b[:]
                            )
                            nc.vector.tensor_mul(etb[:], eb[:], tb[:])
                            for s, src in (
                                (0, eb), (1, tb), (2, e2b), (3, t2b), (4, etb)
                            ):
                                g, sl = divmod(s, 2)
                                col = (sl * N_CHUNKS + c) * CHUNK
                                nc.tensor.matmul(
                                    psum_stats[
                                        32 * g : 32 * g + 1, col : col + CHUNK
                                    ],
                                    ones_bf[:, 0:1],
                                    src[:, :],
                                    start=False,
                                    stop=True,
                                )

                    if body_mode == "dma_only":
                        # timing probe: no compute happened; emit placeholder
                        # out-DMAs from the last loaded io tile
                        for s in range(NSTATS):
                            nc.sync.dma_start(
                                out_d[s : s + 1, :], e_t[0:1, 0:D]
                            )
                    else:
                        # PSUM -> SBUF staging (DMA cannot read PSUM). Split
                        # the free range across DVE and ACT so they overlap.
                        sb_stats = ep.tile(
                            [65, 2 * N_CHUNKS * CHUNK], f32, tag="sb_stats"
                        )
                        nc.vector.tensor_copy(
                            sb_stats[:, : N_CHUNKS * CHUNK],
                            psum_stats[:, : N_CHUNKS * CHUNK],
                        )
                        nc.scalar.copy(
                            sb_stats[:, N_CHUNKS * CHUNK :],
                            psum_stats[:, N_CHUNKS * CHUNK :],
                        )

                        # stats layout: stat s -> partition 32*(s//2),
                        # cols [(s%2)*2048, +2048). Write the 5 stat rows out.
                        for s in range(NSTATS):
                            g, sl = divmod(s, 2)
                            nc.sync.dma_start(
                                out_d[s : s + 1, :],
                                sb_stats[
                                    32 * g : 32 * g + 1, sl * D : (sl + 1) * D
                                ],
                            )

    nc.compile()
    return nc


class _Exec:
    """Cached PJRT executable (shard_map over 8 cores, no collectives)."""

    def __init__(self, nc):
        import jax
        from jax.experimental.shard_map import shard_map
        from jax.sharding import Mesh, PartitionSpec

        from concourse import bass2jax, mybir

        bass2jax.install_neuronx_cc_hook()
        self.nc = nc
        partition_name = (
            nc.partition_id_tensor.name if nc.partition_id_tensor else None
        )

        in_names, out_names, out_avals, zero_outs = [], [], [], []
        for alloc in nc.m.functions[0].allocations:
            if not isinstance(alloc, mybir.MemoryLocationSet):
                continue
            assert alloc.memorylocations
            name = alloc.memorylocations[0].name
            if alloc.kind == "ExternalInput":
                if name != partition_name:
                    in_names.append(name)
            elif alloc.kind == "ExternalOutput":
                shape = tuple(alloc.tensor_shape)
                dtype = mybir.dt.np(alloc.dtype)
                out_names.append(name)
                out_avals.append(jax.core.ShapedArray(shape, dtype))
                zero_outs.append(np.zeros(shape, dtype))

        self.in_names = list(in_names)
        self.out_names = list(out_names)
        self.out_avals = out_avals
        self.zero_outs = zero_outs
        n_params = len(in_names)
        n_outs = len(out_names)

        in_names_full = list(in_names) + list(out_names)
        if partition_name is not None:
            in_names_full.append(partition_name)

        def _body(*args):
            operands = list(args)
            if partition_name is not None:
                operands.append(bass2jax.partition_id_tensor())
            outs = bass2jax._bass_exec_p.bind(
                *operands,
                out_avals=tuple(out_avals),
                in_names=tuple(in_names_full),
                out_names=tuple(out_names),
                lowering_input_output_aliases=(),
                sim_require_finite=True,
                sim_require_nnan=True,
                nc=nc,
            )
            return tuple(outs)

        devices = jax.devices()[:N_CORES]
        assert len(devices) == N_CORES, f"need {N_CORES} devices, got {len(devices)}"
        self.mesh = Mesh(np.asarray(devices), ("core",))
        in_specs = (PartitionSpec("core"),) * (n_params + n_outs)
        out_specs = (PartitionSpec("core"),) * n_outs
        donate = tuple(range(n_params, n_params + n_outs))
        self.sharded = jax.jit(
            shard_map(
                _body,
                mesh=self.mesh,
                in_specs=in_specs,
                out_specs=out_specs,
                check_rep=False,
            ),
            donate_argnums=donate,
            keep_unused=True,
        )

    def concat_zeros(self):
        return [
            np.zeros((N_CORES * z.shape[0], *z.shape[1:]), z.dtype)
            for z in self.zero_outs
        ]

    def run(self, in_map):
        ins = [in_map[name] for name in self.in_names]
        outs = self.sharded(*ins, *self.concat_zeros())
        return {
            name: np.asarray(outs[i]).reshape(
                N_CORES, *self.out_avals[i].shape
            )
            for i, name in enumerate(self.out_names)
        }


def _get_exec(repeat=1, loop=None, **kw):
    key = ("exec", repeat, loop, tuple(sorted(kw.items())))
    if key not in _CACHE:
        _CACHE[key] = _Exec(_build_nc(repeat, loop=loop, **kw))
    return _CACHE[key]


def _host_epilogue(stats):
    """stats: [5, D] float64 global sums -> scalar loss (the unshard step)."""
    n = N_FULL
    s1e, s1t, s2e, s2t, set_ = stats
    var_e = (s2e - s1e * s1e / n) / (n - 1)
    var_t = (s2t - s1t * s1t / n) / (n - 1)
    std_e = np.maximum(np.sqrt(np.maximum(var_e, 0.0)), EPS)
    std_t = np.maximum(np.sqrt(np.maximum(var_t, 0.0)), EPS)
    cov = set_ - s1e * s1t / n
    c = cov / (std_e * std_t) / (n + EPS)
    c = np.clip(c, -1.0 + EPS, 1.0 - EPS)
    return np.sum((1.0 - c) ** 2)


def kernel(e_q, tau):
    e_q = np.ascontiguousarray(np.asarray(e_q), dtype=np.float32)
    tau = np.ascontiguousarray(np.asarray(tau), dtype=np.float32)
    assert e_q.shape == (N_FULL, D) and tau.shape == (N_FULL, D)
    ex = _get_exec()
    # row-sharding across cores: the concatenation of the 8 shards along
    # axis 0 is the full array, so pass it through unchanged.
    outs = ex.run({"e_q": e_q, "tau": tau})
    # each core holds partial sums over its 2048 rows; summing across cores
    # and running the tiny epilogue is the gather/unshard step.
    stats = outs["out"].astype(np.float64).sum(axis=0)
    loss = _host_epilogue(stats)
    return np.asarray(loss, dtype=np.float32)
